# revision 6
# baseline (speedup 1.0000x reference)
"""Multi-head attention (B=4, S=2048, H=16, D=64) on 8 TRN2 NeuronCores.

Strategy: 64 independent (b, h) attention slices, 8 per core, no
cross-core communication.  Per slice, per 512-wide query block:
  mm1 (PE, fp32r):  S^T[k,i] = K^T_tile.T @ Q^T_block      (contract d=64)
  exp (ACT):        E^T = exp(0.125 * S^T)   PSUM -> SBUF  (softmax scale)
  mm2 (PE, fp32r):  Oext^T[0:65,i] += Vext_tile.T @ E^T_tile (contract k)
      where Vext = [V | ones] so row 64 of Oext^T is the softmax row-sum.
  norm (DVE + PE):  recip of row-sum, broadcast via ones-matmul,
                    multiply, DMA out (output stays transposed [d, i];
                    host reassembles the [B,S,H,D] layout).

Inputs are rearranged on the host into per-slice transposed layouts so
all device DMAs are contiguous.
"""

import numpy as np

import concourse.bass as bass
import concourse.mybir as mybir
import concourse.tile as tile
from concourse import bacc
from concourse.bass_utils import run_bass_kernel_spmd

B, S, H, D = 4, 2048, 16, 64
N_CORES = 8
SLICES = B * H              # 64 independent attention slices
SPC = SLICES // N_CORES     # 8 slices per core
KT = S // 128               # 16 key tiles of 128
IBLK = 512                  # query block width (fp32 moving-operand max)
NIB = S // IBLK             # 4 query blocks per slice
F32 = mybir.dt.float32
F32R = mybir.dt.float32r
BF16 = mybir.dt.bfloat16

# k-tile groups: mm1 writes a [128, 512*len] PSUM stage, one ACT exp per
# group.  3-bank groups keep PSUM at 2*3 (stage ping/pong) + 2 (acc) = 8.
GROUPS = [(0, 3), (3, 3), (6, 3), (9, 3), (12, 3), (15, 1)]

_CACHE = {}


def _build():
    nc = bacc.Bacc("TRN2", target_bir_lowering=False, debug=False)

    qt_d = nc.declare_dram_parameter("qt", [SPC, 128, S], BF16, isOutput=False).ap()
    kt_d = nc.declare_dram_parameter("kt", [SPC, 128, S], BF16, isOutput=False).ap()
    vx_d = nc.declare_dram_parameter("vx", [SPC, 128, KT, 65], BF16, isOutput=False).ap()
    out_d = nc.declare_dram_parameter("out", [SPC, D, S], F32, isOutput=True).ap()

    EXP = mybir.ActivationFunctionType.Exp
    MUL = mybir.AluOpType.mult

    with tile.TileContext(nc) as tc:
        with (
            tc.tile_pool(name="qk", bufs=2) as qk_pool,
            tc.tile_pool(name="vp", bufs=2) as v_pool,
            tc.tile_pool(name="et", bufs=4) as e_pool,
            tc.tile_pool(name="sm", bufs=3) as sm_pool,
            tc.tile_pool(name="ones", bufs=1) as ones_pool,
            tc.tile_pool(name="stg", bufs=2, space="PSUM") as stg_pool,
            tc.tile_pool(name="acc", bufs=2, space="PSUM") as acc_pool,
        ):
            ones_sb = ones_pool.tile([1, D], BF16)
            nc.vector.memset(ones_sb[:], 1.0)

            # Deferred normalization state: emitted two pipeline steps
            # later so the PE's broadcast matmul never heads the queue
            # while the DVE reciprocal chain is still running.
            pending = []

            def emit_norm_pe(acc, rr_sb, o_tmp, s, i0):
                # Broadcast recip row-sum across d partitions: K=1 matmul
                # ones[1,64].T @ rr[1,512] -> acc rows 64:128 (reusing the
                # accumulator bank's empty partitions).
                nc.tensor.matmul(
                    acc[D : 2 * D, :],
                    lhsT=ones_sb[:],
                    rhs=rr_sb[:],
                    start=True,
                    stop=True,
                )
                o_sb = sm_pool.tile([D, IBLK], F32, tag="osb")
                nc.vector.tensor_tensor(o_sb[:], o_tmp[:], acc[D : 2 * D, :], MUL)
                nc.sync.dma_start(out_d[s, :, i0 : i0 + IBLK], o_sb[:])

            for s in range(SPC):
                qt_sb = qk_pool.tile([128, S], BF16, tag="qt")
                kt_sb = qk_pool.tile([128, S], BF16, tag="kt")
                vx_sb = v_pool.tile([128, KT, 65], BF16, tag="vx")
                nc.sync.dma_start(qt_sb[:], qt_d[s])
                nc.sync.dma_start(kt_sb[:], kt_d[s])
                nc.sync.dma_start(vx_sb[:], vx_d[s])

                for ib in range(NIB):
                    i0 = ib * IBLK
                    acc = acc_pool.tile([128, IBLK], F32, tag="acc")
                    ready = []  # exp'd groups awaiting their mm2

                    def emit_mm2(acc=None):
                        et, p0, pl = ready.pop(0)
                        for j in range(pl):
                            k = p0 + j
                            nc.tensor.matmul(
                                acc[0 : D + 1, :],
                                lhsT=vx_sb[:, k, :],
                                rhs=et[:, j * IBLK : (j + 1) * IBLK],
                                start=(k == 0),
                                stop=(k == KT - 1),
                            )

                    for gi, (g0, gl) in enumerate(GROUPS):
                        w = gl * IBLK
                        stg = stg_pool.tile([128, 3 * IBLK], F32, tag="stg")
                        for j in range(gl):
                            k = g0 + j
                            p0 = (k % 2) * D  # alternate row halves globally
                            nc.tensor.matmul(
                                stg[:, j * IBLK : (j + 1) * IBLK],
                                lhsT=kt_sb[p0 : p0 + D, k * 128 : (k + 1) * 128],
                                rhs=qt_sb[p0 : p0 + D, i0 : i0 + IBLK],
                                start=True,
                                stop=True,
                            )
                        # PE epilogue: mm2 runs two groups behind its exp
                        if gi >= 2:
                            emit_mm2(acc)
                        if gi == 1 and pending:
                            emit_norm_pe(*pending.pop())
                        et = e_pool.tile([128, 3 * IBLK], BF16, tag="et")
                        nc.scalar.activation(et[:, :w], stg[:, :w], EXP, scale=0.125)
                        ready.append((et, g0, gl))

                    while ready:
                        emit_mm2(acc)

                    # DVE part of the normalization (PE part is deferred)
                    r_sb = sm_pool.tile([1, IBLK], F32, tag="rsb")
                    nc.vector.tensor_copy(r_sb[:], acc[D : D + 1, :])
                    rr_f = sm_pool.tile([1, IBLK], F32, tag="rrf")
                    nc.vector.reciprocal_approx_fast(rr_f[:], r_sb[:])
                    rr_sb = sm_pool.tile([1, IBLK], BF16, tag="rrsb")
                    nc.vector.tensor_copy(rr_sb[:], rr_f[:])
                    o_tmp = sm_pool.tile([D, IBLK], F32, tag="otmp")
                    nc.vector.tensor_copy(o_tmp[:], acc[0:D, :])
                    pending.append((acc, rr_sb, o_tmp, s, i0))

            while pending:
                emit_norm_pe(*pending.pop())

    nc.compile()
    return nc


import ml_dtypes  # noqa: E402

BF16_NP = ml_dtypes.bfloat16


def _prep(qw, kw, vw):
    """Host-side layout prep: per-slice transposed views, contiguous."""
    qw = np.asarray(qw, dtype=np.float32)
    kw = np.asarray(kw, dtype=np.float32)
    vw = np.asarray(vw, dtype=np.float32)

    def to_t(x):  # [B, S, H*D] -> [SLICES, D, S]
        x4 = x.reshape(B, S, H, D)
        return np.ascontiguousarray(
            x4.transpose(0, 2, 3, 1).reshape(SLICES, D, S)
        )

    qt = to_t(qw).astype(BF16_NP)
    kt = to_t(kw).astype(BF16_NP)
    qt = np.ascontiguousarray(np.concatenate([qt, qt], axis=1))  # [SLICES,128,S]
    kt = np.ascontiguousarray(np.concatenate([kt, kt], axis=1))
    v4 = vw.reshape(B, S, H, D).transpose(0, 2, 1, 3)  # [B, H, S, D]
    v5 = v4.reshape(SLICES, KT, 128, D)
    vx = np.empty((SLICES, KT, 128, 65), BF16_NP)
    vx[..., :D] = v5.astype(BF16_NP)
    vx[..., D] = 1.0
    vx = np.ascontiguousarray(vx.transpose(0, 2, 1, 3))  # [SLICES, 128, KT, 65]
    return qt, kt, vx


def kernel(qw, kw, vw):
    if "nc" not in _CACHE:
        _CACHE["nc"] = _build()
    nc = _CACHE["nc"]

    qt, kt, vx = _prep(qw, kw, vw)
    in_maps = [
        {
            "qt": qt[c * SPC : (c + 1) * SPC],
            "kt": kt[c * SPC : (c + 1) * SPC],
            "vx": vx[c * SPC : (c + 1) * SPC],
        }
        for c in range(N_CORES)
    ]
    res = run_bass_kernel_spmd(nc, in_maps, core_ids=list(range(N_CORES)))
    outs = np.stack([np.asarray(res.results[c]["out"]) for c in range(N_CORES)])
    # [N_CORES, SPC, D, S] -> [B, H, D, S] -> [B, S, H, D]
    o = outs.reshape(B, H, D, S).transpose(0, 3, 1, 2)
    return np.ascontiguousarray(o)


# revision 7
# speedup vs baseline: 1.2076x; 1.2076x over previous
"""Multi-head attention (B=4, S=2048, H=16, D=64) on 8 TRN2 NeuronCores.

Strategy: 64 independent (b, h) attention slices, 8 per core, no
cross-core communication.  Per slice, per 512-wide query block:
  mm1 (PE, fp32r):  S^T[k,i] = K^T_tile.T @ Q^T_block      (contract d=64)
  exp (ACT):        E^T = exp(0.125 * S^T)   PSUM -> SBUF  (softmax scale)
  mm2 (PE, fp32r):  Oext^T[0:65,i] += Vext_tile.T @ E^T_tile (contract k)
      where Vext = [V | ones] so row 64 of Oext^T is the softmax row-sum.
  norm (DVE + PE):  recip of row-sum, broadcast via ones-matmul,
                    multiply, DMA out (output stays transposed [d, i];
                    host reassembles the [B,S,H,D] layout).

Inputs are rearranged on the host into per-slice transposed layouts so
all device DMAs are contiguous.
"""

import numpy as np

import concourse.bass as bass
import concourse.mybir as mybir
import concourse.tile as tile
from concourse import bacc
from concourse.bass_utils import run_bass_kernel_spmd

B, S, H, D = 4, 2048, 16, 64
N_CORES = 8
SLICES = B * H              # 64 independent attention slices
SPC = SLICES // N_CORES     # 8 slices per core
KT = S // 128               # 16 key tiles of 128
IBLK = 512                  # query block width (fp32 moving-operand max)
NIB = S // IBLK             # 4 query blocks per slice
F32 = mybir.dt.float32
F32R = mybir.dt.float32r
BF16 = mybir.dt.bfloat16

# k-tile groups: mm1 writes a [128, 512*len] PSUM stage, one ACT exp per
# group.  3-bank groups keep PSUM at 2*3 (stage ping/pong) + 2 (acc) = 8.
GROUPS = [(k, 2) for k in range(0, 16, 2)]

_CACHE = {}


def _build():
    nc = bacc.Bacc("TRN2", target_bir_lowering=False, debug=False)

    qt_d = nc.declare_dram_parameter("qt", [SPC, 128, S], BF16, isOutput=False).ap()
    kt_d = nc.declare_dram_parameter("kt", [SPC, 128, S], BF16, isOutput=False).ap()
    vx_d = nc.declare_dram_parameter("vx", [SPC, 128, KT, 65], BF16, isOutput=False).ap()
    out_d = nc.declare_dram_parameter("out", [SPC, D, S], F32, isOutput=True).ap()

    EXP = mybir.ActivationFunctionType.Exp
    MUL = mybir.AluOpType.mult

    with tile.TileContext(nc) as tc:
        with (
            tc.tile_pool(name="qk", bufs=2) as qk_pool,
            tc.tile_pool(name="vp", bufs=2) as v_pool,
            tc.tile_pool(name="et", bufs=4) as e_pool,
            tc.tile_pool(name="sm", bufs=3) as sm_pool,
            tc.tile_pool(name="ones", bufs=1) as ones_pool,
            tc.tile_pool(name="stg", bufs=3, space="PSUM") as stg_pool,
            tc.tile_pool(name="acc", bufs=2, space="PSUM") as acc_pool,
        ):
            ones_sb = ones_pool.tile([1, D], BF16)
            nc.vector.memset(ones_sb[:], 1.0)

            # Deferred normalization state: emitted two pipeline steps
            # later so the PE's broadcast matmul never heads the queue
            # while the DVE reciprocal chain is still running.
            pending = []

            def emit_norm_pe(acc, rr_sb, o_tmp, s, i0):
                # Broadcast recip row-sum across d partitions: K=1 matmul
                # ones[1,64].T @ rr[1,512] -> acc rows 64:128 (reusing the
                # accumulator bank's empty partitions).
                nc.tensor.matmul(
                    acc[D : 2 * D, :],
                    lhsT=ones_sb[:],
                    rhs=rr_sb[:],
                    start=True,
                    stop=True,
                )
                o_sb = sm_pool.tile([D, IBLK], F32, tag="osb")
                nc.vector.tensor_tensor(o_sb[:], o_tmp[:], acc[D : 2 * D, :], MUL)
                nc.sync.dma_start(out_d[s, :, i0 : i0 + IBLK], o_sb[:])

            for s in range(SPC):
                qt_sb = qk_pool.tile([128, S], BF16, tag="qt")
                kt_sb = qk_pool.tile([128, S], BF16, tag="kt")
                vx_sb = v_pool.tile([128, KT, 65], BF16, tag="vx")
                nc.sync.dma_start(qt_sb[:], qt_d[s])
                nc.sync.dma_start(kt_sb[:], kt_d[s])
                nc.sync.dma_start(vx_sb[:], vx_d[s])

                for ib in range(NIB):
                    i0 = ib * IBLK
                    acc = acc_pool.tile([128, IBLK], F32, tag="acc")
                    ready = []  # exp'd groups awaiting their mm2

                    def emit_mm2(acc=None):
                        et, p0, pl = ready.pop(0)
                        for j in range(pl):
                            k = p0 + j
                            nc.tensor.matmul(
                                acc[0 : D + 1, :],
                                lhsT=vx_sb[:, k, :],
                                rhs=et[:, j * IBLK : (j + 1) * IBLK],
                                start=(k == 0),
                                stop=(k == KT - 1),
                            )

                    for gi, (g0, gl) in enumerate(GROUPS):
                        w = gl * IBLK
                        stg = stg_pool.tile([128, 2 * IBLK], F32, tag="stg")
                        for j in range(gl):
                            k = g0 + j
                            p0 = (j % 2) * D  # row half: partitions 0-63 / 64-127
                            nc.tensor.matmul(
                                stg[:, j * IBLK : (j + 1) * IBLK],
                                lhsT=kt_sb[p0 : p0 + D, k * 128 : (k + 1) * 128],
                                rhs=qt_sb[p0 : p0 + D, i0 : i0 + IBLK],
                                start=True,
                                stop=True,
                            )
                        # PE epilogue: mm2 runs two groups behind its exp
                        if gi >= 2:
                            emit_mm2(acc)
                        if gi == 1 and pending:
                            emit_norm_pe(*pending.pop())
                        et = e_pool.tile([128, 2 * IBLK], BF16, tag="et")
                        nc.scalar.activation(et[:, :w], stg[:, :w], EXP, scale=0.125)
                        ready.append((et, g0, gl))

                    while ready:
                        emit_mm2(acc)

                    # DVE part of the normalization (PE part is deferred)
                    r_sb = sm_pool.tile([1, IBLK], F32, tag="rsb")
                    nc.vector.tensor_copy(r_sb[:], acc[D : D + 1, :])
                    rr_f = sm_pool.tile([1, IBLK], F32, tag="rrf")
                    nc.vector.reciprocal_approx_fast(rr_f[:], r_sb[:])
                    rr_sb = sm_pool.tile([1, IBLK], BF16, tag="rrsb")
                    nc.vector.tensor_copy(rr_sb[:], rr_f[:])
                    o_tmp = sm_pool.tile([D, IBLK], F32, tag="otmp")
                    nc.vector.tensor_copy(o_tmp[:], acc[0:D, :])
                    pending.append((acc, rr_sb, o_tmp, s, i0))

            while pending:
                emit_norm_pe(*pending.pop())

    nc.compile()
    return nc


import ml_dtypes  # noqa: E402

BF16_NP = ml_dtypes.bfloat16


def _prep(qw, kw, vw):
    """Host-side layout prep: per-slice transposed views, contiguous."""
    qw = np.asarray(qw, dtype=np.float32)
    kw = np.asarray(kw, dtype=np.float32)
    vw = np.asarray(vw, dtype=np.float32)

    def to_t(x):  # [B, S, H*D] -> [SLICES, D, S]
        x4 = x.reshape(B, S, H, D)
        return np.ascontiguousarray(
            x4.transpose(0, 2, 3, 1).reshape(SLICES, D, S)
        )

    qt = to_t(qw).astype(BF16_NP)
    kt = to_t(kw).astype(BF16_NP)
    qt = np.ascontiguousarray(np.concatenate([qt, qt], axis=1))  # [SLICES,128,S]
    kt = np.ascontiguousarray(np.concatenate([kt, kt], axis=1))
    v4 = vw.reshape(B, S, H, D).transpose(0, 2, 1, 3)  # [B, H, S, D]
    v5 = v4.reshape(SLICES, KT, 128, D)
    vx = np.empty((SLICES, KT, 128, 65), BF16_NP)
    vx[..., :D] = v5.astype(BF16_NP)
    vx[..., D] = 1.0
    vx = np.ascontiguousarray(vx.transpose(0, 2, 1, 3))  # [SLICES, 128, KT, 65]
    return qt, kt, vx


def kernel(qw, kw, vw):
    if "nc" not in _CACHE:
        _CACHE["nc"] = _build()
    nc = _CACHE["nc"]

    qt, kt, vx = _prep(qw, kw, vw)
    in_maps = [
        {
            "qt": qt[c * SPC : (c + 1) * SPC],
            "kt": kt[c * SPC : (c + 1) * SPC],
            "vx": vx[c * SPC : (c + 1) * SPC],
        }
        for c in range(N_CORES)
    ]
    res = run_bass_kernel_spmd(nc, in_maps, core_ids=list(range(N_CORES)))
    outs = np.stack([np.asarray(res.results[c]["out"]) for c in range(N_CORES)])
    # [N_CORES, SPC, D, S] -> [B, H, D, S] -> [B, S, H, D]
    o = outs.reshape(B, H, D, S).transpose(0, 3, 1, 2)
    return np.ascontiguousarray(o)


# revision 8
# speedup vs baseline: 1.2115x; 1.0032x over previous
"""Multi-head attention (B=4, S=2048, H=16, D=64) on 8 TRN2 NeuronCores.

Strategy: 64 independent (b, h) attention slices, 8 per core, no
cross-core communication.  Per slice, per 512-wide query block:
  mm1 (PE, fp32r):  S^T[k,i] = K^T_tile.T @ Q^T_block      (contract d=64)
  exp (ACT):        E^T = exp(0.125 * S^T)   PSUM -> SBUF  (softmax scale)
  mm2 (PE, fp32r):  Oext^T[0:65,i] += Vext_tile.T @ E^T_tile (contract k)
      where Vext = [V | ones] so row 64 of Oext^T is the softmax row-sum.
  norm (DVE + PE):  recip of row-sum, broadcast via ones-matmul,
                    multiply, DMA out (output stays transposed [d, i];
                    host reassembles the [B,S,H,D] layout).

Inputs are rearranged on the host into per-slice transposed layouts so
all device DMAs are contiguous.
"""

import numpy as np

import concourse.bass as bass
import concourse.mybir as mybir
import concourse.tile as tile
from concourse import bacc
from concourse.bass_utils import run_bass_kernel_spmd

B, S, H, D = 4, 2048, 16, 64
N_CORES = 8
SLICES = B * H              # 64 independent attention slices
SPC = SLICES // N_CORES     # 8 slices per core
KT = S // 128               # 16 key tiles of 128
IBLK = 512                  # query block width (fp32 moving-operand max)
NIB = S // IBLK             # 4 query blocks per slice
F32 = mybir.dt.float32
F32R = mybir.dt.float32r
BF16 = mybir.dt.bfloat16

# k-tile groups: mm1 writes a [128, 512*len] PSUM stage, one ACT exp per
# group.  3-bank groups keep PSUM at 2*3 (stage ping/pong) + 2 (acc) = 8.
GROUPS = [(k, 2) for k in range(0, 16, 2)]

_CACHE = {}


def _build():
    nc = bacc.Bacc("TRN2", target_bir_lowering=False, debug=False)

    qt_d = nc.declare_dram_parameter("qt", [SPC, 128, S], BF16, isOutput=False).ap()
    kt_d = nc.declare_dram_parameter("kt", [SPC, 128, S], BF16, isOutput=False).ap()
    vx_d = nc.declare_dram_parameter("vx", [SPC, 128, KT, 65], BF16, isOutput=False).ap()
    out_d = nc.declare_dram_parameter("out", [SPC, D, S], F32, isOutput=True).ap()

    EXP = mybir.ActivationFunctionType.Exp
    MUL = mybir.AluOpType.mult

    with tile.TileContext(nc) as tc:
        with (
            tc.tile_pool(name="qk", bufs=2) as qk_pool,
            tc.tile_pool(name="vp", bufs=2) as v_pool,
            tc.tile_pool(name="et", bufs=4) as e_pool,
            tc.tile_pool(name="sm", bufs=4) as sm_pool,
            tc.tile_pool(name="ones", bufs=1) as ones_pool,
            tc.tile_pool(name="stg", bufs=3, space="PSUM") as stg_pool,
            tc.tile_pool(name="acc", bufs=2, space="PSUM") as acc_pool,
        ):
            ones_sb = ones_pool.tile([1, D], BF16)
            nc.vector.memset(ones_sb[:], 1.0)

            # Deferred normalization state: emitted two pipeline steps
            # later so the PE's broadcast matmul never heads the queue
            # while the DVE reciprocal chain is still running.
            pending = []

            def emit_norm_pe(acc, rr_sb, o_tmp, s, i0):
                # Broadcast recip row-sum across d partitions: K=1 matmul
                # ones[1,64].T @ rr[1,512] -> acc rows 64:128 (reusing the
                # accumulator bank's empty partitions).
                nc.tensor.matmul(
                    acc[D : 2 * D, :],
                    lhsT=ones_sb[:],
                    rhs=rr_sb[:],
                    start=True,
                    stop=True,
                )
                o_sb = sm_pool.tile([D, IBLK], F32, tag="osb")
                nc.vector.tensor_tensor(o_sb[:], o_tmp[:], acc[D : 2 * D, :], MUL)
                nc.sync.dma_start(out_d[s, :, i0 : i0 + IBLK], o_sb[:])

            for s in range(SPC):
                qt_sb = qk_pool.tile([128, S], BF16, tag="qt")
                kt_sb = qk_pool.tile([128, S], BF16, tag="kt")
                vx_sb = v_pool.tile([128, KT, 65], BF16, tag="vx")
                nc.sync.dma_start(qt_sb[:], qt_d[s])
                nc.sync.dma_start(kt_sb[:], kt_d[s])
                nc.sync.dma_start(vx_sb[:], vx_d[s])

                for ib in range(NIB):
                    i0 = ib * IBLK
                    acc = acc_pool.tile([128, IBLK], F32, tag="acc")
                    ready = []  # exp'd groups awaiting their mm2

                    def emit_mm2(acc=None):
                        et, p0, pl = ready.pop(0)
                        for j in range(pl):
                            k = p0 + j
                            nc.tensor.matmul(
                                acc[0 : D + 1, :],
                                lhsT=vx_sb[:, k, :],
                                rhs=et[:, j * IBLK : (j + 1) * IBLK],
                                start=(k == 0),
                                stop=(k == KT - 1),
                            )

                    for gi, (g0, gl) in enumerate(GROUPS):
                        w = gl * IBLK
                        stg = stg_pool.tile([128, 2 * IBLK], F32, tag="stg")
                        for j in range(gl):
                            k = g0 + j
                            p0 = (j % 2) * D  # row half: partitions 0-63 / 64-127
                            nc.tensor.matmul(
                                stg[:, j * IBLK : (j + 1) * IBLK],
                                lhsT=kt_sb[p0 : p0 + D, k * 128 : (k + 1) * 128],
                                rhs=qt_sb[p0 : p0 + D, i0 : i0 + IBLK],
                                start=True,
                                stop=True,
                            )
                        # PE epilogue: mm2 runs two groups behind its exp
                        if gi >= 2:
                            emit_mm2(acc)
                        if gi == 3 and pending:
                            emit_norm_pe(*pending.pop())
                        et = e_pool.tile([128, 2 * IBLK], BF16, tag="et")
                        nc.scalar.activation(et[:, :w], stg[:, :w], EXP, scale=0.125)
                        ready.append((et, g0, gl))

                    while ready:
                        emit_mm2(acc)

                    # DVE part of the normalization (PE part is deferred)
                    r_sb = sm_pool.tile([1, IBLK], F32, tag="rsb")
                    nc.vector.tensor_copy(r_sb[:], acc[D : D + 1, :])
                    rr_f = sm_pool.tile([1, IBLK], F32, tag="rrf")
                    nc.vector.reciprocal_approx_fast(rr_f[:], r_sb[:])
                    rr_sb = sm_pool.tile([1, IBLK], BF16, tag="rrsb")
                    nc.vector.tensor_copy(rr_sb[:], rr_f[:])
                    o_tmp = sm_pool.tile([D, IBLK], F32, tag="otmp")
                    nc.vector.tensor_copy(o_tmp[:], acc[0:D, :])
                    pending.append((acc, rr_sb, o_tmp, s, i0))

            while pending:
                emit_norm_pe(*pending.pop())

    nc.compile()
    return nc


import ml_dtypes  # noqa: E402

BF16_NP = ml_dtypes.bfloat16


def _prep(qw, kw, vw):
    """Host-side layout prep: per-slice transposed views, contiguous."""
    qw = np.asarray(qw, dtype=np.float32)
    kw = np.asarray(kw, dtype=np.float32)
    vw = np.asarray(vw, dtype=np.float32)

    def to_t(x):  # [B, S, H*D] -> [SLICES, D, S]
        x4 = x.reshape(B, S, H, D)
        return np.ascontiguousarray(
            x4.transpose(0, 2, 3, 1).reshape(SLICES, D, S)
        )

    qt = to_t(qw).astype(BF16_NP)
    kt = to_t(kw).astype(BF16_NP)
    qt = np.ascontiguousarray(np.concatenate([qt, qt], axis=1))  # [SLICES,128,S]
    kt = np.ascontiguousarray(np.concatenate([kt, kt], axis=1))
    v4 = vw.reshape(B, S, H, D).transpose(0, 2, 1, 3)  # [B, H, S, D]
    v5 = v4.reshape(SLICES, KT, 128, D)
    vx = np.empty((SLICES, KT, 128, 65), BF16_NP)
    vx[..., :D] = v5.astype(BF16_NP)
    vx[..., D] = 1.0
    vx = np.ascontiguousarray(vx.transpose(0, 2, 1, 3))  # [SLICES, 128, KT, 65]
    return qt, kt, vx


def kernel(qw, kw, vw):
    if "nc" not in _CACHE:
        _CACHE["nc"] = _build()
    nc = _CACHE["nc"]

    qt, kt, vx = _prep(qw, kw, vw)
    in_maps = [
        {
            "qt": qt[c * SPC : (c + 1) * SPC],
            "kt": kt[c * SPC : (c + 1) * SPC],
            "vx": vx[c * SPC : (c + 1) * SPC],
        }
        for c in range(N_CORES)
    ]
    res = run_bass_kernel_spmd(nc, in_maps, core_ids=list(range(N_CORES)))
    outs = np.stack([np.asarray(res.results[c]["out"]) for c in range(N_CORES)])
    # [N_CORES, SPC, D, S] -> [B, H, D, S] -> [B, S, H, D]
    o = outs.reshape(B, H, D, S).transpose(0, 3, 1, 2)
    return np.ascontiguousarray(o)
